# revision 25
# baseline (speedup 1.0000x reference)
"""AdaFS (top-k field-selection MLP) on Trainium2, 8 NeuronCores,
pure data parallel (2048 of 16384 batch rows per core).

Math (per batch row, matching the jax reference):
  flat = field.reshape(B, 2560)                  # col d*5+f
  logits = MLP_ctrl(flat)                        # 2560 -> 64 -> 32 -> 5
  keep the top-3 fields of softmax(logits); softmax is monotone, so the
  selection runs on the logits with lowest-index tie break (matching
  jax.lax.top_k), and the kept weights are renormalized:
      mask_f = ind_f * exp(l_f) / sum_g ind_g exp(l_g)
  out = MLP_main(flat * mask_per_field)          # 2560 -> 1280 -> 5 -> 1

Device kernel, per core: 4 super-tiles of 512 rows, software-pipelined
one super deep so the Tensor engine never waits on the controller:

  - x is transposed to field-major [2560, 2048] on the HOST and split
    into bf16 hi + lo streams (host prep is layout/cast only); xh rides
    the Scalar engine's DMA ring, xl the GpSimd ring, weights/outputs
    the Sync ring, so the streams don't head-of-line block each other
  - controller needs fp32-accurate logits (top3/top4 gaps go below
    1e-5), so layer 1 runs as three bf16 matmuls Wh.xh + Wl.xh + Wh.xl
    accumulated in fp32 PSUM - error ~2^-18.  The 40 controller matmuls
    for super s+1 are interleaved two-per-two into the first half of
    super s's main-MLP passes, which both hides their PE time behind
    the mask-dependency latency and spreads their x-stream DMA over a
    ~30us window instead of a burst
  - main MLP layer 1 (the dominant 200 matmuls/super) runs k-outer in
    5 passes of 2 PSUM chains each, so pass 0 only needs xtm[k=0] to
    start (not all 20 k-tiles) and PSUM stays within 8 banks:
    4 main + 1 c1 + 1 c3 + 1 small + 1 mask-broadcast.  mW1 is loaded
    in pass-major chunks so each pass chases its own 1.3MB stream, and
    all weights are host-prelaid as SBUF images (large-row DMAs: the
    per-k / rearranged loads previously choked the rings with 7000+
    sub-1KB packets for ~65us)
  - top-3 selection in [128 batch, 5] layout via pairwise logit
    compares with a stable lowest-index tie-break, exp on Scalar,
    renormalization with nc.vector.reciprocal (emitted during super
    s's passes 2-3 so it overlaps PE work)
  - the [5, 512] mask is broadcast across partitions by a
    selector-matrix matmul on the PE; the mask multiply then runs
    IN PLACE on the xh tile (the controller for that super has already
    consumed the unmasked values), saving 40KB/partition of SBUF
  - main MLP in bf16 (weights pre-permuted field-major and pre-cast on
    the host), 512-wide moving operands, fp32 PSUM accumulation; the
    mW2/oW tail of super s is interleaved into pass 0 of super s+1
"""

from contextlib import ExitStack

import numpy as np
import ml_dtypes

import concourse.bass as bass
import concourse.mybir as mybir
import concourse.tile as tile
from concourse.bass_utils import run_bass_kernel_spmd
from concourse.vector_clock import ScopedClock

F32 = mybir.dt.float32
BF16 = mybir.dt.bfloat16
AF = mybir.ActivationFunctionType
ALU = mybir.AluOpType
AX = mybir.AxisListType

B, D, F = 16384, 512, 5
E = D * F  # 2560
H1 = E // 2  # 1280
NK = E // 128  # 20 feature k-tiles
NN = H1 // 128  # 10 hidden n-tiles
SUP = 512  # batch rows per super-tile
NSUB = SUP // 128  # 4 subtiles
NCORES = 8
B_CORE = B // NCORES  # 2048


class _TC(tile.TileContext):
    """TileContext that limits every instruction to one semaphore wait
    (this walrus build rejects multi-wait instructions): extra waits are
    hoisted onto same-engine NOPs inserted just before the instruction."""

    def _add_instruction(self, inst):
        si = getattr(inst, "sync_info", None)
        if si is not None and si.on_wait and len(si.on_wait) > 1:
            waits = list(si.on_wait)
            for w in waits[:-1]:
                nop = mybir.InstNoOp(
                    name=self.nc.get_next_instruction_name(),
                    sync_info=mybir.SyncInfo(on_wait=[w], on_update=[]),
                    engine=inst.engine,
                    bass_nofuse=True,
                )
                super()._add_instruction(nop)
            inst.sync_info = mybir.SyncInfo(
                on_wait=waits[-1:], on_update=list(si.on_update or [])
            )
        super()._add_instruction(inst)

    def _drain_and_barrier(self, tick_clock, wait_clock):
        drain_inst = self.nc.sync.drain()
        wait_clock.add_sem_waits(
            drain_inst.ins, ScopedClock({None: tick_clock.global_clock})
        )
        si = drain_inst.ins.sync_info
        if si is not None and si.on_wait and len(si.on_wait) > 1:
            waits = list(si.on_wait)
            si.on_wait = waits[:1]
            for i in range(1, len(waits)):
                extra = self.nc.sync.drain()
                extra.ins.sync_info = type(si)(on_wait=[waits[i]], on_update=[])
        self.nc.all_engine_barrier()
        assert self.sems is not None
        popped = self.nc._tile_sem_poison_stack.pop()
        assert popped is self._sem_poison
        self.nc.clear_and_free_semaphores(list(self.sems.allocated().values()))
        self.nc.all_engine_barrier()


def build_nc(b_core=B_CORE):
    nsup = b_core // SUP
    nc = bass.Bass()
    dp = nc.declare_dram_parameter
    xh_d = dp("xh", [E, b_core], BF16, isOutput=False)
    xl_d = dp("xl", [E, b_core], BF16, isOutput=False)
    cW1p_d = dp("cW1p", [128, NK * 128], BF16, isOutput=False)
    cW1h_d = dp("cW1h", [128, NK * 64], BF16, isOutput=False)
    cW2_d = dp("cW2", [64, 32], F32, isOutput=False)
    cW3_d = dp("cW3", [32, F], F32, isOutput=False)
    cb1_d = dp("cb1", [64, 1], F32, isOutput=False)
    cb2_d = dp("cb2", [32, 1], F32, isOutput=False)
    cb3_d = dp("cb3", [F, 1], F32, isOutput=False)
    mW1_d = dp("mW1", [F * 128, NK * 256], BF16, isOutput=False)
    mb1_d = dp("mb1", [128, NN], F32, isOutput=False)
    mW2_d = dp("mW2", [128, NN * F], BF16, isOutput=False)
    mb2_d = dp("mb2", [F, 1], F32, isOutput=False)
    oW_d = dp("oW", [F, 1], BF16, isOutput=False)
    ob_d = dp("ob", [1, 1], F32, isOutput=False)
    eye_d = dp("eye", [128, 128], F32, isOutput=False)
    sel_d = dp("sel", [F, F * 128], BF16, isOutput=False)
    lt_d = dp("lt", [128, NSUB * F * F], F32, isOutput=False)
    out_d = dp("out", [b_core, 1], F32, isOutput=True)

    with _TC(nc) as tc, ExitStack() as ctx:
        constp = ctx.enter_context(tc.tile_pool(name="const", bufs=1))
        xhp = ctx.enter_context(tc.tile_pool(name="xh", bufs=2))
        xlp = ctx.enter_context(tc.tile_pool(name="xl", bufs=2))
        h1p = ctx.enter_context(tc.tile_pool(name="h1", bufs=2))
        smallp = ctx.enter_context(tc.tile_pool(name="small", bufs=1))
        pmain = ctx.enter_context(tc.tile_pool(name="pm", bufs=4, space="PSUM"))
        pc1p = ctx.enter_context(tc.tile_pool(name="pc1", bufs=1, space="PSUM"))
        pc3p = ctx.enter_context(tc.tile_pool(name="pc3", bufs=1, space="PSUM"))
        psmp = ctx.enter_context(tc.tile_pool(name="psm", bufs=1, space="PSUM"))
        pbcp = ctx.enter_context(tc.tile_pool(name="pbc", bufs=1, space="PSUM"))

        # ---- persistent weights/constants (host-prepared SBUF images,
        # one large-row DMA each so no ring chokes on micro-packets) ----
        cW1psb = constp.tile([128, NK * 128], BF16)
        cW1hsb = constp.tile([128, NK * 64], BF16)
        nc.sync.dma_start(cW1psb[:], cW1p_d[:])
        nc.gpsimd.dma_start(cW1hsb[:], cW1h_d[:])
        cW2sb = constp.tile([64, 32], F32)
        nc.sync.dma_start(cW2sb[:], cW2_d[:])
        cW3sb = constp.tile([32, F], F32)
        nc.sync.dma_start(cW3sb[:], cW3_d[:])
        cb1sb = constp.tile([64, 1], F32)
        nc.sync.dma_start(cb1sb[:], cb1_d[:])
        cb2sb = constp.tile([32, 1], F32)
        nc.sync.dma_start(cb2sb[:], cb2_d[:])
        cb3sb = constp.tile([F, 1], F32)
        nc.sync.dma_start(cb3sb[:], cb3_d[:])
        mb1sb = constp.tile([128, NN], F32)
        nc.sync.dma_start(mb1sb[:], mb1_d[:])
        mW2sb = constp.tile([128, NN * F], BF16)
        nc.sync.dma_start(mW2sb[:], mW2_d[:])
        mb2sb = constp.tile([F, 1], F32)
        nc.sync.dma_start(mb2sb[:], mb2_d[:])
        oWsb = constp.tile([F, 1], BF16)
        nc.sync.dma_start(oWsb[:], oW_d[:])
        obsb = constp.tile([1, 1], F32)
        nc.sync.dma_start(obsb[:], ob_d[:])
        eyesb = constp.tile([128, 128], F32)
        nc.sync.dma_start(eyesb[:], eye_d[:])
        selsb = constp.tile([F, F * 128], BF16)
        nc.sync.dma_start(selsb[:], sel_d[:])
        ltsb = constp.tile([128, NSUB * F * F], F32)
        nc.sync.dma_start(ltsb[:], lt_d[:])
        mW1sb = constp.tile([128, NK * H1], BF16)

        def mW1_chunk(p, ring):
            # pass-major weight layout: host row block p is the SBUF image
            # of pass p's stationary weights; four DMAs per pass so the
            # main k-loop can chase the stream at 5-k-tile granularity
            for q in range(4):
                c0 = q * (NK // 4) * 256
                ring.dma_start(
                    mW1sb[:, p * NK * 256 + c0 : p * NK * 256 + c0 + (NK // 4) * 256],
                    mW1_d[p * 128 : (p + 1) * 128, c0 : c0 + (NK // 4) * 256],
                )


        # ---- per-super state ----
        xt = [None] * (nsup + 2)
        xl = [None] * (nsup + 2)
        c1ps = [None] * (nsup + 1)
        c3ps = [None] * (nsup + 1)
        lT = [None] * (nsup + 1)
        m20 = [None] * (nsup + 1)
        mtb = [None] * (nsup + 1)
        h1t = [None] * (nsup + 1)
        h2ps = [None] * (nsup + 1)
        h2r = [None] * (nsup + 1)
        ops = [None] * (nsup + 1)

        def alloc_x(s):
            # supers 0-1 (the DMA-bound ramp, no WAR deps yet): alternate
            # rings by k parity so neither stream alone paces the
            # controller.  Later supers: dedicated rings, so a WAR-stalled
            # xh descriptor can never head-of-line block the xl stream.
            xt[s] = xhp.tile([128, NK * SUP], BF16, tag="xh", name=f"xt{s}")
            xl[s] = xlp.tile([128, NK * SUP], BF16, tag="xl", name=f"xl{s}")
            for k in range(NK):
                swap = s < 2 and k % 2 == 1
                (nc.gpsimd if swap else nc.scalar).dma_start(
                    xt[s][:, k * SUP : (k + 1) * SUP],
                    xh_d[k * 128 : (k + 1) * 128, s * SUP : (s + 1) * SUP],
                )
                (nc.scalar if swap else nc.gpsimd).dma_start(
                    xl[s][:, k * SUP : (k + 1) * SUP],
                    xl_d[k * 128 : (k + 1) * 128, s * SUP : (s + 1) * SUP],
                )

        def ctrl_pair(s, k):
            if k == 0:
                c1ps[s] = pc1p.tile([128, SUP], F32, tag="c1ps", name=f"c1ps{s}")
                c3ps[s] = pc3p.tile([64, SUP], F32, tag="c3ps", name=f"c3ps{s}")
            nc.tensor.matmul(
                c1ps[s][:],
                cW1psb[:, k * 128 : (k + 1) * 128],
                xt[s][:, k * SUP : (k + 1) * SUP],
                start=(k == 0),
                stop=(k == NK - 1),
            )
            nc.tensor.matmul(
                c3ps[s][:],
                cW1hsb[:, k * 64 : (k + 1) * 64],
                xl[s][:, k * SUP : (k + 1) * SUP],
                start=(k == 0),
                stop=(k == NK - 1),
            )

        def ctrl_tail_a(s):
            # c1 = relu(Wh.xh + Wl.xh + Wh.xl + cb1); packed pass left
            # Wh.xh in psum rows 0:64 and Wl.xh in rows 64:128
            hi64 = smallp.tile([64, SUP], F32, tag="hi64")
            nc.vector.tensor_copy(hi64[:], c1ps[s][64:128, :])
            t2 = smallp.tile([64, SUP], F32, tag="t2")
            nc.vector.tensor_add(t2[:], c1ps[s][0:64, :], hi64[:])
            nc.vector.tensor_add(t2[:], t2[:], c3ps[s][:])
            c1 = smallp.tile([64, SUP], F32, tag="c1")
            nc.scalar.activation(c1[:], t2[:], AF.Relu, bias=cb1sb[:, 0:1])
            c2ps = psmp.tile([32, SUP], F32, tag="psm")
            nc.tensor.matmul(c2ps[:], cW2sb[:], c1[:], start=True, stop=True)
            c2 = smallp.tile([32, SUP], F32, tag="c2")
            nc.scalar.activation(c2[:], c2ps[:], AF.Relu, bias=cb2sb[:, 0:1])
            return c2

        def ctrl_tail_b(s, c2):
            lps = psmp.tile([F, SUP], F32, tag="psm")
            nc.tensor.matmul(lps[:], cW3sb[:], c2[:], start=True, stop=True)
            lT[s] = smallp.tile([F, SUP], F32, tag="lT", name=f"lT{s}")
            nc.scalar.activation(lT[s][:], lps[:], AF.Identity, bias=cb3sb[:, 0:1])
            # logits to batch-partition layout [128, 4*5]
            ltp = psmp.tile([128, NSUB * F], F32, tag="psm")
            for j in range(NSUB):
                nc.tensor.transpose(
                    ltp[:, j * F : (j + 1) * F],
                    lT[s][:, j * 128 : (j + 1) * 128],
                    eyesb[0:F, 0:F],
                )
            l_bt = smallp.tile([128, NSUB * F], F32, tag="l_bt")
            nc.vector.tensor_copy(l_bt[:], ltp[:])
            # top-3 mask, stable ties (count of strictly-greater plus
            # lower-index-equal entries < 3)
            e_bt = smallp.tile([128, NSUB * F], F32, tag="e_bt")
            nc.scalar.activation(e_bt[:], l_bt[:], AF.Exp)
            lv = l_bt[:].rearrange("p (j f) -> p j f", f=F)
            a_v = lv.unsqueeze(3).broadcast_to([128, NSUB, F, F])
            b_v = lv.unsqueeze(2).broadcast_to([128, NSUB, F, F])
            g4 = smallp.tile([128, NSUB * F * F], F32, tag="g4")
            gv = g4[:].rearrange("p (j f g) -> p j f g", f=F, g=F)
            nc.vector.tensor_tensor(gv, b_v, a_v, ALU.is_gt)
            e4 = smallp.tile([128, NSUB * F * F], F32, tag="e4")
            ev = e4[:].rearrange("p (j f g) -> p j f g", f=F, g=F)
            nc.vector.tensor_tensor(ev, b_v, a_v, ALU.is_equal)
            nc.vector.tensor_mul(e4[:], e4[:], ltsb[:])
            nc.vector.tensor_add(g4[:], g4[:], e4[:])
            cnt = smallp.tile([128, NSUB * F], F32, tag="cnt")
            nc.vector.tensor_reduce(
                cnt[:],
                g4[:].rearrange("p (jf g) -> p jf g", g=F),
                AX.X,
                ALU.add,
            )
            ind = smallp.tile([128, NSUB * F], F32, tag="ind")
            nc.vector.tensor_single_scalar(ind[:], cnt[:], 2.5, ALU.is_lt)
            w20 = smallp.tile([128, NSUB * F], F32, tag="w20")
            nc.vector.tensor_mul(w20[:], ind[:], e_bt[:])
            s4 = smallp.tile([128, NSUB], F32, tag="s4")
            nc.vector.tensor_reduce(
                s4[:], w20[:].rearrange("p (j f) -> p j f", f=F), AX.X, ALU.add
            )
            r4 = smallp.tile([128, NSUB], F32, tag="r4")
            nc.vector.reciprocal(r4[:], s4[:])
            m20[s] = smallp.tile([128, NSUB * F], F32, tag="m20", name=f"m20_{s}")
            nc.vector.tensor_tensor(
                m20[s][:].rearrange("p (j f) -> p j f", f=F),
                w20[:].rearrange("p (j f) -> p j f", f=F),
                r4[:].unsqueeze(2).broadcast_to([128, NSUB, F]),
                ALU.mult,
            )

        def mask_transpose(s):
            # mask back to [5, 512] bf16
            mtp = psmp.tile([F, SUP], F32, tag="psm")
            for j in range(NSUB):
                nc.tensor.transpose(
                    mtp[:, j * 128 : (j + 1) * 128],
                    m20[s][:, j * F : (j + 1) * F],
                    eyesb[:],
                )
            mtb[s] = smallp.tile([F, SUP], BF16, tag="mtb", name=f"mtb{s}")
            nc.vector.tensor_copy(mtb[s][:], mtp[:])

        def sel_field(s, f):
            # broadcast mask row f across partitions via selector matmul,
            # drain to SBUF bf16, then apply IN PLACE to the four xh
            # k-tiles of field f
            pbc = pbcp.tile([128, SUP], F32, tag="pbc")
            nc.tensor.matmul(
                pbc[:],
                selsb[:, f * 128 : (f + 1) * 128],
                mtb[s][:],
                start=True,
                stop=True,
            )
            mbc = smallp.tile([128, SUP], BF16, tag=f"mbc{f % 2}", name=f"mbc{s}_{f}")
            nc.scalar.activation(mbc[:], pbc[:], AF.Identity)
            for jj in range(4):
                k = f * 4 + jj
                nc.vector.tensor_mul(
                    xt[s][:, k * SUP : (k + 1) * SUP],
                    xt[s][:, k * SUP : (k + 1) * SUP],
                    mbc[:],
                )

        def mm_tail_step(s, step):
            # mW2/oW tail of super s, fed piecewise into pass 0 of s+1
            if step < NN:
                if step == 0:
                    h2ps[s] = psmp.tile([F, SUP], F32, tag="psm", name=f"h2ps{s}")
                nc.tensor.matmul(
                    h2ps[s][:],
                    mW2sb[:, step * F : (step + 1) * F],
                    h1t[s][:, step * SUP : (step + 1) * SUP],
                    start=(step == 0),
                    stop=(step == NN - 1),
                )
                if step == NN - 1:
                    h2r[s] = smallp.tile([F, SUP], BF16, tag="h2r", name=f"h2r{s}")
                    nc.scalar.activation(
                        h2r[s][:], h2ps[s][:], AF.Relu, bias=mb2sb[:, 0:1]
                    )
            elif step == NN + 1:
                ops[s] = psmp.tile([1, SUP], F32, tag="psm", name=f"ops{s}")
                nc.tensor.matmul(
                    ops[s][:], oWsb[:], h2r[s][:], start=True, stop=True
                )
                osb = smallp.tile([1, SUP], F32, tag="osb")
                nc.scalar.activation(osb[:], ops[s][:], AF.Identity, bias=obsb[:, 0:1])
                nc.sync.dma_start(
                    out_d[s * SUP : (s + 1) * SUP, 0:1].rearrange("b one -> one b"),
                    osb[:],
                )

        def main_super(s):
            h1t[s] = h1p.tile([128, NN * SUP], BF16, tag="h1t", name=f"h1t{s}")
            c2_next = None
            for p in range(5):
                na, nb = 2 * p, 2 * p + 1
                pa = pmain.tile([128, SUP], F32, tag="pm")
                pb = pmain.tile([128, SUP], F32, tag="pm")
                for k in range(NK):
                    for n, pt in ((na, pa), (nb, pb)):
                        off = p * NK * 256 + k * 256 + (n % 2) * 128
                        nc.tensor.matmul(
                            pt[:],
                            mW1sb[:, off : off + 128],
                            xt[s][:, k * SUP : (k + 1) * SUP],
                            start=(k == 0),
                            stop=(k == NK - 1),
                        )
                    if p < 2 and s + 1 < nsup and k % 4 == 0:
                        ctrl_pair(s + 1, p * 10 + k // 2)
                        ctrl_pair(s + 1, p * 10 + k // 2 + 1)
                    if p == 1 and s > 0 and k == 2:
                        mm_tail_step(s - 1, NN + 1)
                    if p == 4 and s + 1 < nsup and k % 4 == 0:
                        sel_field(s + 1, k // 4)
                    if p == 4 and s == nsup - 1 and k >= 12:
                        mm_tail_step(s, k - 12)
                if p == 0 and s > 0:
                    # previous super's mW2 tail, burst after pass 0
                    for step in range(NN):
                        mm_tail_step(s - 1, step)
                if s + 1 < nsup:
                    if p == 1:
                        c2_next = ctrl_tail_a(s + 1)
                    elif p == 2:
                        ctrl_tail_b(s + 1, c2_next)
                    elif p == 3:
                        mask_transpose(s + 1)
                for n, pt in ((na, pa), (nb, pb)):
                    nc.scalar.activation(
                        h1t[s][:, n * SUP : (n + 1) * SUP],
                        pt[:],
                        AF.Relu,
                        bias=mb1sb[:, n : n + 1],
                    )

        # ---- prologue: super 0's controller runs unpipelined.  Ring FIFO
        # does the prioritization: x(0) stripes sit ahead of mW1 chunks,
        # which sit ahead of x(1), on all three rings ----
        alloc_x(0)
        for p in range(5):
            mW1_chunk(p, nc.sync)
        for k in range(NK):
            ctrl_pair(0, k)
        c2_0 = ctrl_tail_a(0)
        ctrl_tail_b(0, c2_0)
        mask_transpose(0)
        alloc_x(1)
        for f in range(F):
            sel_field(0, f)

        for s in range(nsup):
            main_super(s)
            if s + 2 < nsup:
                alloc_x(s + 2)
        # last super's tail: steps 0..7 rode in pass 4; finish the rest
        s = nsup - 1
        for step in range(8, NN + 2):
            mm_tail_step(s, step)

    return nc


def _host_arrays(inputs, b_core=B_CORE):
    """Prepare per-core input maps from the full problem inputs."""
    bf16 = ml_dtypes.bfloat16
    f32 = np.float32

    def fm(w):  # interleaved (d*5+f) rows -> field-major (f*512+d) rows
        return np.ascontiguousarray(
            w.reshape(D, F, -1).transpose(1, 0, 2).reshape(E, -1)
        )

    field = np.asarray(inputs["field"], f32)
    flat = field.reshape(field.shape[0], E)
    cW1fm = fm(np.asarray(inputs["cW1"], f32))
    cW1h = cW1fm.astype(bf16)
    cW1l = (cW1fm - cW1h.astype(f32)).astype(bf16)

    def sbuf_image(w):  # [E, C] k-sliced -> [128, NK*C] SBUF image
        c = w.shape[1]
        return np.ascontiguousarray(
            w.reshape(NK, 128, c).transpose(1, 0, 2).reshape(128, NK * c)
        )

    shared = {
        "cW1p": sbuf_image(np.concatenate([cW1h, cW1l], axis=1)),
        "cW1h": sbuf_image(cW1h),
        "cW2": np.ascontiguousarray(np.asarray(inputs["cW2"], f32)),
        "cW3": np.ascontiguousarray(np.asarray(inputs["cW3"], f32)),
        "cb1": np.asarray(inputs["cb1"], f32).reshape(64, 1),
        "cb2": np.asarray(inputs["cb2"], f32).reshape(32, 1),
        "cb3": np.asarray(inputs["cb3"], f32).reshape(F, 1),
        "mW1": np.ascontiguousarray(
            fm(np.asarray(inputs["mW1"], f32))
            .astype(bf16)
            .reshape(NK, 128, F, 256)
            .transpose(2, 1, 0, 3)
            .reshape(F * 128, NK * 256)
        ),
        "mb1": np.ascontiguousarray(
            np.asarray(inputs["mb1"], f32).reshape(NN, 128).T
        ),
        "mW2": np.ascontiguousarray(
            np.asarray(inputs["mW2"], f32)
            .astype(bf16)
            .reshape(NN, 128, F)
            .transpose(1, 0, 2)
            .reshape(128, NN * F)
        ),
        "mb2": np.asarray(inputs["mb2"], f32).reshape(F, 1),
        "oW": np.ascontiguousarray(np.asarray(inputs["oW"], f32)).astype(bf16),
        "ob": np.asarray(inputs["ob"], f32).reshape(1, 1),
        "eye": np.eye(128, dtype=f32),
        "sel": np.ascontiguousarray(
            np.repeat(np.eye(F, dtype=bf16), 128, axis=1)
        ),
        "lt": np.ascontiguousarray(
            np.broadcast_to(
                np.tril(np.ones((F, F), f32), -1), (128, NSUB, F, F)
            ).reshape(128, NSUB * F * F)
        ),
    }
    perm = (np.arange(D)[None, :] * F + np.arange(F)[:, None]).reshape(-1)
    ncores = flat.shape[0] // b_core
    in_maps = []
    for c in range(ncores):
        m = dict(shared)
        xt = np.ascontiguousarray(flat[c * b_core : (c + 1) * b_core][:, perm].T)
        xh = xt.astype(bf16)
        m["xh"] = xh
        m["xl"] = (xt - xh.astype(f32)).astype(bf16)
        in_maps.append(m)
    return in_maps


_NC_CACHE = {}


def _get_nc(b_core=B_CORE):
    if b_core not in _NC_CACHE:
        _NC_CACHE[b_core] = build_nc(b_core)
    return _NC_CACHE[b_core]


def run(inputs, trace=False):
    nc = _get_nc(B_CORE)
    in_maps = _host_arrays(inputs, B_CORE)
    res = run_bass_kernel_spmd(
        nc, in_maps, core_ids=list(range(NCORES)), trace=trace
    )
    out = np.concatenate(
        [res.results[c]["out"] for c in range(NCORES)], axis=0
    ).astype(np.float32)
    return out, res


def kernel(**inputs):
    out, _ = run(inputs, trace=False)
    return out
